# revision 23
# baseline (speedup 1.0000x reference)
# Trainium2 Bass kernel for nn_BDH_66056597013022 (dense_transformer).
#
# Model (per reference):
#   v = LN(emb_w[tokens])                                  [B,T,D]
#   6x: x  = relu(v @ Dx_h)            per head            [B,H,T,Dh]
#       xr = RoPE(x)
#       S  = xr @ xr^T                 (no softmax)        [B,H,T,T]
#       a  = S @ v                                         [B,H,T,D]
#       y  = relu(a @ Dy_h) * x                            [B,H,T,Dh]
#       v  = LN(v + LN(concat_h(y) @ E))
#   out = v @ readout                                      [B,T,V]
#
# Shapes: B=4 T=1024 H=4 N=4096 D=256 L=6 V=256, Dh=N/H=1024.
#
# Sharding (8 cores): core c -> batch b=c//2, head-pair hp=c%2 (heads 2hp,2hp+1).
# Cross-core coupling is only the head-sum of z = y @ E: a 2-rank AllReduce
# per layer between cores {2b,2b+1}, split into two token-half AllReduces so
# the second half's collective overlaps the first half's v-update and the
# next layer's x-projection.
#
# dtypes: v/vT, S, aT, Dx, Dy, readout are float32r (TF32-like, 1 cyc/row);
# xT/xr (RoPE path), y, z/AllReduce path, E, cos/sin, embedding are bf16.
# bf16 gives the DVE 2x packed mode for the RoPE tensor_tensor chain (the
# scores input only; the residual/v chain stays f32r for accuracy) and
# halves the collective payload. PSUM accumulation is fp32 everywhere.
#
# Per-layer pipeline (emission = intended engine order):
#   scores h0 + aT h0 (PE)      [RoPE h1 of this layer ran during prev tail]
#   y/z h0 (PE; STT relu*x on DVE), z0T kept in SBUF
#   scores h1 + aT h1, y/z h1 in token halves; each half: fused add(z0T)+
#     evict -> DMA -> AllReduce (pair) -> DMA back
#   per half: transpose z back to [T,D], LN chain, v update, transpose_v,
#     then NEXT layer's x-proj for that token half + RoPE (so the second
#     half's AllReduce hides behind the first half's x-proj/RoPE).

import os
import numpy as np

B, T, H, N, D, L, V = 4, 1024, 4, 4096, 256, 6, 256
Dh = N // H
EPS = 1e-5
NCORES = 8
P = 128
NT = T // P    # 8 token tiles
ND = D // P    # 2 model-dim tiles
NDh = Dh // P  # 8 head-dim tiles
HF = T // 2    # 512 token half

_CACHE = {}
LAST_RESULT = None


def _build_program():
    from contextlib import ExitStack

    import concourse.bass as bass
    import concourse.bacc as bacc
    import concourse.tile as tile
    import concourse.mybir as mybir
    from concourse.masks import make_identity

    f32 = mybir.dt.float32
    f32r = mybir.dt.float32r
    bf16 = mybir.dt.bfloat16
    AF = mybir.ActivationFunctionType
    ALU = mybir.AluOpType
    ts = bass.ts

    nc = bacc.Bacc("TRN2", target_bir_lowering=False, debug=False,
                   enable_asserts=False, num_devices=NCORES)

    d_oh = nc.dram_tensor("onehotT", [V, T], bf16, kind="ExternalInput").ap()
    d_ew = nc.dram_tensor("emb_w", [V, D], bf16, kind="ExternalInput").ap()
    d_dx = nc.dram_tensor("dx", [2 * D, Dh], bf16, kind="ExternalInput").ap()
    d_dy = nc.dram_tensor("dy", [2 * D, Dh], bf16, kind="ExternalInput").ap()
    d_eh = nc.dram_tensor("eh", [2 * Dh, D], bf16, kind="ExternalInput").ap()
    d_cos = nc.dram_tensor("cosT", [Dh // 2, T], bf16, kind="ExternalInput").ap()
    d_sin = nc.dram_tensor("sinT", [Dh // 2, T], bf16, kind="ExternalInput").ap()
    d_ro = nc.dram_tensor("readout", [D, V], bf16, kind="ExternalInput").ap()
    d_out = nc.dram_tensor("out", [T, V], f32, kind="ExternalOutput").ap()

    with tile.TileContext(nc) as tc, ExitStack() as ctx:
        wpool = ctx.enter_context(tc.tile_pool(name="weights", bufs=1))
        vpool = ctx.enter_context(tc.tile_pool(name="vpool", bufs=1))
        xpool = ctx.enter_context(tc.tile_pool(name="xpool", bufs=1))
        xrpool = ctx.enter_context(tc.tile_pool(name="xrpool", bufs=1))
        spool = ctx.enter_context(tc.tile_pool(name="spool", bufs=4))
        apool = ctx.enter_context(tc.tile_pool(name="apool", bufs=2))
        ypool = ctx.enter_context(tc.tile_pool(name="ypool", bufs=6))
        zpool = ctx.enter_context(tc.tile_pool(name="zpool", bufs=1))
        zqpool = ctx.enter_context(tc.tile_pool(name="zqpool", bufs=4))
        lnpool = ctx.enter_context(tc.tile_pool(name="lnpool", bufs=4))
        stpool = ctx.enter_context(tc.tile_pool(name="stpool", bufs=6))
        rtpool = ctx.enter_context(tc.tile_pool(name="rtpool", bufs=4))
        psA = ctx.enter_context(tc.tile_pool(name="psA", bufs=4, space="PSUM"))
        psB = ctx.enter_context(tc.tile_pool(name="psB", bufs=2, space="PSUM"))
        dpool = ctx.enter_context(tc.tile_pool(name="drampool", bufs=2, space="DRAM"))

        # ---- persistent weights ----
        def load_bf(dram_ap, n_tiles, width, tag):
            tiles = []
            for i in range(n_tiles):
                t = wpool.tile([P, width], bf16, tag=f"{tag}{i}", name=f"{tag}{i}")
                nc.sync.dma_start(t[:], dram_ap[ts(i, P), :])
                tiles.append(t)
            return tiles

        # embedding inputs first so its matmuls start while the big weight
        # DMAs are still in flight
        oh_sb = []
        for k in range(ND):
            t = spool.tile([P, T], bf16, tag="score", name=f"oh{k}")
            nc.sync.dma_start(t[:], d_oh[ts(k, P), :])
            oh_sb.append(t)
        ew_sb = load_bf(d_ew, ND, D, "ew")
        dx_sb = load_bf(d_dx, 4, Dh, "dx")
        cos_sb = load_bf(d_cos, 4, T, "cos")
        sin_sb = load_bf(d_sin, 4, T, "sin")
        dy_sb = load_bf(d_dy, 4, Dh, "dy")
        eh_sb = load_bf(d_eh, 16, D, "eh")
        ro_sb = load_bf(d_ro, ND, V, "ro")

        identf = wpool.tile([P, P], f32, tag="identf", name="identf")
        make_identity(nc, identf)
        identr = wpool.tile([P, P], f32r, tag="identr", name="identr")
        nc.scalar.copy(identr[:], identf[:])
        identb = wpool.tile([P, P], bf16, tag="identb", name="identb")
        nc.scalar.copy(identb[:], identf[:])
        epsc = wpool.tile([P, 1], f32, tag="epsc", name="epsc")
        nc.gpsimd.memset(epsc[:], EPS)
        warmsink = wpool.tile([P, 1], f32, tag="warmsink", name="warmsink")

        def keep_warm(n_mms, label):
            # dependency-free matmuls to hold the PE clock at 2.4 GHz across
            # a known stall (the AllReduce tail / LN chains leave PE idle
            # long enough for the HAM to re-throttle to 1.2 GHz otherwise)
            wps = psA.tile([P, 512], f32, tag="psA", name=f"warm_{label}")
            for i in range(n_mms):
                nc.tensor.matmul(wps[:], dx_sb[0][:, 0:P], dx_sb[1][:, 0:512],
                                 start=(i == 0), stop=(i == n_mms - 1))
            nc.vector.tensor_copy(warmsink[:], wps[:, 0:1])

        # ---- persistent activations ----
        v_sb = [vpool.tile([P, D], f32r, tag=f"v{m}", name=f"v{m}")
                for m in range(NT)]
        vT_sb = [vpool.tile([P, T], bf16, tag=f"vT{k}", name=f"vT{k}")
                 for k in range(ND)]
        # xT/xr: 2 heads x 8 Dh-tiles, persistent slots reused per layer
        xT = [[xpool.tile([P, T], bf16, tag=f"xT{j}_{e}", name=f"xT{j}_{e}")
               for e in range(NDh)] for j in range(2)]
        xr = [[xrpool.tile([P, T], bf16, tag=f"xr{j}_{e}", name=f"xr{j}_{e}")
               for e in range(NDh)] for j in range(2)]

        def ln_stats(src_ap, want_nmr):
            # rstd (and optionally -mean*rstd) of src over the free dim
            st6 = stpool.tile([P, 6], f32, tag="st6", name="st6")
            nc.vector.bn_stats(st6[:], src_ap)
            mv = stpool.tile([P, 2], f32, tag="mv", name="mv")
            nc.vector.bn_aggr(mv[:], st6[:])
            sd = stpool.tile([P, 1], f32, tag="sd", name="sd")
            nc.scalar.activation(sd[:], mv[:, 1:2], AF.Sqrt, bias=epsc[:], scale=1.0)
            rstd = stpool.tile([P, 1], f32, tag="rstd", name="rstd")
            nc.vector.reciprocal(rstd[:], sd[:])
            if not want_nmr:
                return rstd, None
            nmr = stpool.tile([P, 1], f32, tag="nmr", name="nmr")
            nc.vector.scalar_tensor_tensor(
                nmr[:], mv[:, 0:1], -1.0, rstd[:], op0=ALU.mult, op1=ALU.mult)
            return rstd, nmr

        def layer_norm(src_ap, dst_ap):
            rstd, nmr = ln_stats(src_ap, True)
            nc.scalar.activation(dst_ap, src_ap, AF.Identity,
                                 bias=nmr[:], scale=rstd[:])

        def transpose_v(half):
            for m in range(4 * half, 4 * half + 4):
                for d in range(ND):
                    tps = psA.tile([P, P], f32r, tag="psA", name="tvp")
                    nc.tensor.transpose(tps[:], v_sb[m][:, ts(d, P)], identr[:])
                    nc.vector.tensor_copy(vT_sb[d][:, ts(m, P)], tps[:])  # CAST to bf16

        def emit_xproj(j, half):
            # xT[j][e][:, half] = relu(Dx_j^T @ vT[:, half]); evict alternates
            # ACT/DVE so neither engine paces the 16-matmul burst
            hs = slice(half * HF, (half + 1) * HF)
            for e in range(NDh):
                xps = psA.tile([P, HF], f32, tag="psA", name="xps")
                for k in range(ND):
                    nc.tensor.matmul(xps[:], dx_sb[2 * j + k][:, ts(e, P)],
                                     vT_sb[k][:, hs],
                                     start=(k == 0), stop=(k == ND - 1))
                nc.scalar.activation(xT[j][e][:, hs], xps[:], AF.Relu)

        def emit_rope(j, half):
            # xr = x*cos + rotate_half(x)*sin on [128,512] bf16 slices
            # (all-bf16 SBUF operands -> DVE 2x packed mode)
            hs = slice(half * HF, (half + 1) * HF)
            for m in range(4):
                lo, hi = xT[j][m], xT[j][m + 4]
                xrl, xrh = xr[j][m], xr[j][m + 4]
                cm, sm = cos_sb[m], sin_sb[m]
                t1 = rtpool.tile([P, HF], bf16, tag="rt", name="rt1")
                nc.vector.tensor_mul(t1[:], hi[:, hs], sm[:, hs])
                nc.vector.tensor_mul(xrl[:, hs], lo[:, hs], cm[:, hs])
                nc.vector.tensor_sub(xrl[:, hs], xrl[:, hs], t1[:])
                t2 = rtpool.tile([P, HF], bf16, tag="rt", name="rt2")
                nc.vector.tensor_mul(t2[:], lo[:, hs], sm[:, hs])
                nc.vector.tensor_mul(xrh[:, hs], hi[:, hs], cm[:, hs])
                nc.vector.tensor_add(xrh[:, hs], xrh[:, hs], t2[:])

        def emit_scores_aT(j):
            # S = xr @ xr^T streamed per 128-row tile; aT += v^T @ S with a
            # one-tile lag so the PE never waits on the ACT eviction. S is
            # numerically symmetric so [t,s] tiles serve as [s,t] operands.
            aT_ps = [psB.tile([P, T], f32, tag="psB", name=f"aTps{m}")
                     for m in range(ND)]
            s_tiles = [None] * NT

            def emit_aT(k):
                for m in range(ND):
                    for n in range(2):
                        nc.tensor.matmul(
                            aT_ps[m][:, ts(n, HF)], v_sb[k][:, ts(m, P)],
                            s_tiles[k][:, ts(n, HF)],
                            start=(k == 0), stop=(k == NT - 1))

            for k in range(NT):
                s_sb = spool.tile([P, T], f32r, tag="score", name=f"s{k}")
                for n in range(2):
                    sps = psA.tile([P, HF], f32, tag="psA", name="sps")
                    for kk in range(NDh):
                        nc.tensor.matmul(
                            sps[:], xr[j][kk][:, ts(k, P)],
                            xr[j][kk][:, ts(n, HF)],
                            start=(kk == 0), stop=(kk == NDh - 1))
                    if n == 0:
                        nc.scalar.copy(s_sb[:, ts(n, HF)], sps[:])
                    else:
                        nc.vector.tensor_copy(s_sb[:, ts(n, HF)], sps[:])
                s_tiles[k] = s_sb
                if k > 0:
                    emit_aT(k - 1)
            emit_aT(NT - 1)
            aT = []
            for m in range(ND):
                at = apool.tile([P, T], bf16, tag="aT", name=f"aT{m}")
                aT.append(at)
            # evict in (n, m) order so y's first matmuls (token half 0) start
            # after two of the four half-copies
            for n in range(2):
                for m in range(ND):
                    nc.scalar.copy(aT[m][:, ts(n, HF)], aT_ps[m][:, ts(n, HF)])
            return aT

        def emit_yz_half(j, half, aT, z_ps):
            # yT = relu(Dy^T aT) * xT for one token half; z_ps[:, half]
            # accumulates E_h^T @ yT with a one-tile lag behind the DVE fusion
            hs = slice(half * HF, (half + 1) * HF)
            y_half = [None] * NDh

            def emit_z(kk):
                for m in range(ND):
                    nc.tensor.matmul(
                        z_ps[m][:, hs], eh_sb[8 * j + kk][:, ts(m, P)],
                        y_half[kk][:],
                        start=(kk == 0), stop=(kk == NDh - 1))

            for kk in range(NDh):
                yps = psA.tile([P, HF], f32, tag="psA", name="yps")
                for k in range(ND):
                    nc.tensor.matmul(yps[:], dy_sb[2 * j + k][:, ts(kk, P)],
                                     aT[k][:, hs],
                                     start=(k == 0), stop=(k == ND - 1))
                y_sb = ypool.tile([P, HF], bf16, tag="yT", name=f"y{kk}")
                nc.vector.scalar_tensor_tensor(
                    y_sb[:], yps[:], 0.0, xT[j][kk][:, hs],
                    op0=ALU.max, op1=ALU.mult)
                y_half[kk] = y_sb
                if kk > 0:
                    emit_z(kk - 1)
            emit_z(NDh - 1)

        rg = [[0, 1], [2, 3], [4, 5], [6, 7]]

        # ---- embedding: v0 = LN(onehot @ emb_w) ----
        for m in range(NT):
            eps_t = psA.tile([P, D], f32, tag="psA", name="embps")
            for k in range(ND):
                nc.tensor.matmul(eps_t[:], oh_sb[k][:, ts(m, P)], ew_sb[k][:],
                                 start=(k == 0), stop=(k == ND - 1))
            emb_t = lnpool.tile([P, D], f32, tag="w", name="embt")
            nc.scalar.copy(emb_t[:], eps_t[:])
            layer_norm(emb_t[:], v_sb[m][:])
        transpose_v(0)
        transpose_v(1)
        # layer 0 x-proj + RoPE, head-major so scores h0 unblocks earliest
        emit_xproj(0, 0)
        emit_rope(0, 0)
        emit_xproj(0, 1)
        emit_rope(0, 1)
        emit_xproj(1, 0)
        emit_xproj(1, 1)
        emit_rope(1, 0)
        emit_rope(1, 1)

        for layer in range(L):
            # ---- head 0: scores, aT, y/z; z kept on-chip in bf16 ----
            aT0 = emit_scores_aT(0)
            z0_ps = [psB.tile([P, T], f32, tag="psB", name=f"z0ps{m}")
                     for m in range(ND)]
            emit_yz_half(0, 0, aT0, z0_ps)
            emit_yz_half(0, 1, aT0, z0_ps)
            z0T = [zpool.tile([P, T], bf16, tag=f"z0T{m}", name=f"z0T{m}")
                   for m in range(ND)]
            for m in range(ND):
                nc.scalar.copy(z0T[m][:], z0_ps[m][:])

            # ---- head 1: scores, aT, then y/z per token half; each half's
            # z is fused-added with z0T on eviction and AllReduced over the
            # core pair while later work proceeds ----
            aT1 = emit_scores_aT(1)
            z1_ps = [psB.tile([P, T], f32, tag="psB", name=f"z1ps{m}")
                     for m in range(ND)]
            zin = [None, None]
            zout = [None, None]
            zr = [[None] * ND, [None] * ND]
            for half in range(2):
                hs = slice(half * HF, (half + 1) * HF)
                emit_yz_half(1, half, aT1, z1_ps)
                zin[half] = dpool.tile([P, T], bf16, tag=f"zi{half}",
                                       name=f"zi{half}_{layer}")
                zout[half] = dpool.tile([P, T], bf16, tag=f"zo{half}",
                                        name=f"zo{half}_{layer}")
                for m in range(ND):
                    zq1 = zqpool.tile([P, HF], bf16, tag="zq1",
                                      name=f"zq1_{half}_{m}")
                    nc.vector.scalar_tensor_tensor(
                        zq1[:], z1_ps[m][:, hs], 0.0, z0T[m][:, hs],
                        op0=ALU.add, op1=ALU.add)
                    nc.sync.dma_start(zin[half][:, ts(m, HF)], zq1[:])
                nc.gpsimd.collective_compute(
                    "AllReduce", mybir.AluOpType.add,
                    ins=[zin[half].opt()], outs=[zout[half].opt()],
                    replica_groups=rg)
                for m in range(ND):
                    zrt = zqpool.tile([P, HF], bf16, tag=f"zr{half}{m}",
                                      name=f"zr{half}{m}_{layer}")
                    nc.sync.dma_start(zrt[:], zout[half][:, ts(m, HF)])
                    zr[half][m] = zrt
                if half == 0:
                    keep_warm(20, f"ar{layer}")

            # ---- per token half: v update, then next layer's x-proj/RoPE
            # (or the readout) for that half ----
            for half in range(2):
                for mm in range(4):
                    m = 4 * half + mm
                    q = lnpool.tile([P, D], bf16, tag="zq", name=f"zq{m}")
                    for kd in range(ND):
                        tzp = psA.tile([P, P], bf16, tag="psA", name="tzp")
                        nc.tensor.transpose(
                            tzp[:], zr[half][kd][:, ts(mm, P)], identb[:])
                        nc.vector.tensor_copy(q[:, ts(kd, P)], tzp[:])
                    # v' = LN(v + LN(q)); LN is shift-invariant, so the inner
                    # mean offset can be dropped: w' = q*rstd_q + v differs
                    # from v + LN(q) by a per-token constant only.
                    r_q, _ = ln_stats(q[:], False)
                    wp = lnpool.tile([P, D], f32, tag="w", name=f"w{m}")
                    nc.vector.scalar_tensor_tensor(
                        wp[:], q[:], r_q[:], v_sb[m][:].bitcast(f32),
                        op0=ALU.mult, op1=ALU.add)
                    layer_norm(wp[:], v_sb[m][:])
                    for d in range(ND):
                        tps = psA.tile([P, P], f32r, tag="psA", name="tvp")
                        nc.tensor.transpose(
                            tps[:], v_sb[m][:, ts(d, P)], identr[:])
                        nc.vector.tensor_copy(vT_sb[d][:, ts(m, P)], tps[:])
                    if mm < 3:
                        keep_warm(3, f"ln{layer}_{half}_{mm}")
                if layer == L - 1:
                    for m in range(4 * half, 4 * half + 4):
                        rps = psA.tile([P, V], f32, tag="psA", name="rps")
                        for k in range(ND):
                            nc.tensor.matmul(
                                rps[:], vT_sb[k][:, ts(m, P)], ro_sb[k][:],
                                start=(k == 0), stop=(k == ND - 1))
                        o_sb = lnpool.tile([P, V], f32, tag="o", name=f"o{m}")
                        nc.scalar.copy(o_sb[:], rps[:])
                        nc.sync.dma_start(d_out[ts(m, P), :], o_sb[:])
                else:
                    emit_xproj(0, half)
                    emit_rope(0, half)
                    emit_xproj(1, half)
                    if half == 1:
                        emit_rope(1, 0)
                        emit_rope(1, 1)

    nc.compile()
    return nc


def _get_program():
    if "nc" not in _CACHE:
        _CACHE["nc"] = _build_program()
    return _CACHE["nc"]


def _rope_tables():
    inv = (1.0 / (10000.0 ** (np.arange(0, Dh, 2, dtype=np.float32) / Dh)))
    tt = np.arange(T, dtype=np.float32)
    freqs = np.outer(tt, inv).astype(np.float32)  # [T, Dh/2]
    cosT = np.ascontiguousarray(np.cos(freqs).T, dtype=np.float32)
    sinT = np.ascontiguousarray(np.sin(freqs).T, dtype=np.float32)
    return cosT, sinT


def kernel(**inputs):
    global LAST_RESULT
    from concourse import bass_utils
    import ml_dtypes

    bf = ml_dtypes.bfloat16
    tokens = np.asarray(inputs["tokens"])
    emb_w = np.asarray(inputs["emb_w"], dtype=np.float32).astype(bf)
    E = np.asarray(inputs["E"], dtype=np.float32)
    Dx = np.ascontiguousarray(inputs["Dx"], dtype=np.float32).astype(bf)
    Dy = np.ascontiguousarray(inputs["Dy"], dtype=np.float32).astype(bf)
    readout = np.ascontiguousarray(
        inputs["readout"], dtype=np.float32).astype(bf)

    cosT, sinT = _rope_tables()
    cosT = cosT.astype(bf)
    sinT = sinT.astype(bf)

    in_maps = []
    for c in range(NCORES):
        b, hp = c // 2, c % 2
        oh = np.zeros((V, T), dtype=bf)
        oh[np.asarray(tokens[b], dtype=np.int64), np.arange(T)] = 1.0
        in_maps.append({
            "onehotT": oh,
            "emb_w": emb_w,
            "dx": np.ascontiguousarray(
                Dx[2 * hp:2 * hp + 2].reshape(2 * D, Dh)),
            "dy": np.ascontiguousarray(
                Dy[2 * hp:2 * hp + 2].reshape(2 * D, Dh)),
            "eh": np.ascontiguousarray(
                E[2 * hp * Dh:(2 * hp + 2) * Dh]).astype(bf),
            "cosT": cosT,
            "sinT": sinT,
            "readout": readout,
        })

    nc = _get_program()
    res = bass_utils.run_bass_kernel_spmd(
        nc, in_maps, core_ids=list(range(NCORES)),
        trace=bool(int(os.environ.get("KERNEL_TRACE", "0"))))
    LAST_RESULT = res
    out = np.stack([res.results[2 * b]["out"] for b in range(B)], axis=0)
    return out


# revision 25
# speedup vs baseline: 1.0755x; 1.0755x over previous
# Trainium2 Bass kernel for nn_BDH_66056597013022 (dense_transformer).
#
# Model (per reference):
#   v = LN(emb_w[tokens])                                  [B,T,D]
#   6x: x  = relu(v @ Dx_h)            per head            [B,H,T,Dh]
#       xr = RoPE(x)
#       S  = xr @ xr^T                 (no softmax)        [B,H,T,T]
#       a  = S @ v                                         [B,H,T,D]
#       y  = relu(a @ Dy_h) * x                            [B,H,T,Dh]
#       v  = LN(v + LN(concat_h(y) @ E))
#   out = v @ readout                                      [B,T,V]
#
# Shapes: B=4 T=1024 H=4 N=4096 D=256 L=6 V=256, Dh=N/H=1024.
#
# Sharding (8 cores): core c -> batch b=c//2, head-pair hp=c%2 (heads 2hp,2hp+1).
# Cross-core coupling is only the head-sum of z = y @ E: a 2-rank AllReduce
# per layer between cores {2b,2b+1}, split into two token-half AllReduces so
# the second half's collective overlaps the first half's v-update and the
# next layer's x-projection.
#
# dtypes: v/vT, S, aT, Dx, Dy, readout are float32r (TF32-like, 1 cyc/row);
# xT/xr (RoPE path), y, z/AllReduce path, E, cos/sin, embedding are bf16.
# bf16 gives the DVE 2x packed mode for the RoPE tensor_tensor chain (the
# scores input only; the residual/v chain stays f32r for accuracy) and
# halves the collective payload. PSUM accumulation is fp32 everywhere.
#
# Per-layer pipeline (emission = intended engine order):
#   scores h0 + aT h0 (PE)      [RoPE h1 of this layer ran during prev tail]
#   y/z h0 (PE; STT relu*x on DVE), z0T kept in SBUF
#   scores h1 + aT h1, y/z h1 in token halves; each half: fused add(z0T)+
#     evict -> DMA -> AllReduce (pair) -> DMA back
#   per half: transpose z back to [T,D], LN chain, v update, transpose_v,
#     then NEXT layer's x-proj for that token half + RoPE (so the second
#     half's AllReduce hides behind the first half's x-proj/RoPE).

import os
import numpy as np

B, T, H, N, D, L, V = 4, 1024, 4, 4096, 256, 6, 256
Dh = N // H
EPS = 1e-5
NCORES = 8
P = 128
NT = T // P    # 8 token tiles
ND = D // P    # 2 model-dim tiles
NDh = Dh // P  # 8 head-dim tiles
HF = T // 2    # 512 token half

_CACHE = {}
LAST_RESULT = None


def _build_program():
    from contextlib import ExitStack

    import concourse.bass as bass
    import concourse.bacc as bacc
    import concourse.tile as tile
    import concourse.mybir as mybir
    from concourse.masks import make_identity

    f32 = mybir.dt.float32
    f32r = mybir.dt.float32r
    bf16 = mybir.dt.bfloat16
    AF = mybir.ActivationFunctionType
    ALU = mybir.AluOpType
    ts = bass.ts

    nc = bacc.Bacc("TRN2", target_bir_lowering=False, debug=False,
                   enable_asserts=False, num_devices=NCORES)

    d_oh = nc.dram_tensor("onehotT", [V, T], bf16, kind="ExternalInput").ap()
    d_ew = nc.dram_tensor("emb_w", [V, D], bf16, kind="ExternalInput").ap()
    d_dx = nc.dram_tensor("dx", [2 * D, Dh], bf16, kind="ExternalInput").ap()
    d_dy = nc.dram_tensor("dy", [2 * D, Dh], bf16, kind="ExternalInput").ap()
    d_eh = nc.dram_tensor("eh", [2 * Dh, D], bf16, kind="ExternalInput").ap()
    d_cos = nc.dram_tensor("cosT", [Dh // 2, T], bf16, kind="ExternalInput").ap()
    d_sin = nc.dram_tensor("sinT", [Dh // 2, T], bf16, kind="ExternalInput").ap()
    d_ro = nc.dram_tensor("readout", [D, V], bf16, kind="ExternalInput").ap()
    d_out = nc.dram_tensor("out", [T, V], f32, kind="ExternalOutput").ap()

    with tile.TileContext(nc) as tc, ExitStack() as ctx:
        wpool = ctx.enter_context(tc.tile_pool(name="weights", bufs=1))
        vpool = ctx.enter_context(tc.tile_pool(name="vpool", bufs=1))
        xpool = ctx.enter_context(tc.tile_pool(name="xpool", bufs=1))
        xrpool = ctx.enter_context(tc.tile_pool(name="xrpool", bufs=1))
        spool = ctx.enter_context(tc.tile_pool(name="spool", bufs=4))
        apool = ctx.enter_context(tc.tile_pool(name="apool", bufs=2))
        ypool = ctx.enter_context(tc.tile_pool(name="ypool", bufs=6))
        zpool = ctx.enter_context(tc.tile_pool(name="zpool", bufs=1))
        zqpool = ctx.enter_context(tc.tile_pool(name="zqpool", bufs=4))
        lnpool = ctx.enter_context(tc.tile_pool(name="lnpool", bufs=4))
        stpool = ctx.enter_context(tc.tile_pool(name="stpool", bufs=6))
        rtpool = ctx.enter_context(tc.tile_pool(name="rtpool", bufs=4))
        psA = ctx.enter_context(tc.tile_pool(name="psA", bufs=4, space="PSUM"))
        psB = ctx.enter_context(tc.tile_pool(name="psB", bufs=2, space="PSUM"))
        dpool = ctx.enter_context(tc.tile_pool(name="drampool", bufs=2, space="DRAM"))

        # ---- persistent weights ----
        def load_bf(dram_ap, n_tiles, width, tag):
            tiles = []
            for i in range(n_tiles):
                t = wpool.tile([P, width], bf16, tag=f"{tag}{i}", name=f"{tag}{i}")
                nc.sync.dma_start(t[:], dram_ap[ts(i, P), :])
                tiles.append(t)
            return tiles

        # embedding inputs first so its matmuls start while the big weight
        # DMAs are still in flight
        oh_sb = []
        for k in range(ND):
            t = spool.tile([P, T], bf16, tag="score", name=f"oh{k}")
            nc.sync.dma_start(t[:], d_oh[ts(k, P), :])
            oh_sb.append(t)
        ew_sb = load_bf(d_ew, ND, D, "ew")
        dx_sb = load_bf(d_dx, 4, Dh, "dx")
        cos_sb = load_bf(d_cos, 4, T, "cos")
        sin_sb = load_bf(d_sin, 4, T, "sin")
        dy_sb = load_bf(d_dy, 4, Dh, "dy")
        eh_sb = load_bf(d_eh, 16, D, "eh")
        ro_sb = load_bf(d_ro, ND, V, "ro")

        identf = wpool.tile([P, P], f32, tag="identf", name="identf")
        make_identity(nc, identf)
        identr = wpool.tile([P, P], f32r, tag="identr", name="identr")
        nc.scalar.copy(identr[:], identf[:])
        identb = wpool.tile([P, P], bf16, tag="identb", name="identb")
        nc.scalar.copy(identb[:], identf[:])
        epsc = wpool.tile([P, 1], f32, tag="epsc", name="epsc")
        nc.gpsimd.memset(epsc[:], EPS)
        warmsink = wpool.tile([P, 1], f32, tag="warmsink", name="warmsink")

        def keep_warm(n_mms, label):
            # dependency-free matmuls to hold the PE clock at 2.4 GHz across
            # a known stall (the AllReduce tail / LN chains leave PE idle
            # long enough for the HAM to re-throttle to 1.2 GHz otherwise)
            wps = psA.tile([P, 512], f32, tag="psA", name=f"warm_{label}")
            for i in range(n_mms):
                nc.tensor.matmul(wps[:], dx_sb[0][:, 0:P], dx_sb[1][:, 0:512],
                                 start=(i == 0), stop=(i == n_mms - 1))
            nc.scalar.copy(warmsink[:], wps[:, 0:1])

        # ---- persistent activations ----
        v_sb = [vpool.tile([P, D], f32r, tag=f"v{m}", name=f"v{m}")
                for m in range(NT)]
        vT_sb = [vpool.tile([P, T], bf16, tag=f"vT{k}", name=f"vT{k}")
                 for k in range(ND)]
        # xT/xr: 2 heads x 8 Dh-tiles, persistent slots reused per layer
        xT = [[xpool.tile([P, T], bf16, tag=f"xT{j}_{e}", name=f"xT{j}_{e}")
               for e in range(NDh)] for j in range(2)]
        xr = [[xrpool.tile([P, T], bf16, tag=f"xr{j}_{e}", name=f"xr{j}_{e}")
               for e in range(NDh)] for j in range(2)]

        def ln_stats(src_ap, want_nmr):
            # rstd (and optionally -mean*rstd) of src over the free dim
            st6 = stpool.tile([P, 6], f32, tag="st6", name="st6")
            nc.vector.bn_stats(st6[:], src_ap)
            mv = stpool.tile([P, 2], f32, tag="mv", name="mv")
            nc.vector.bn_aggr(mv[:], st6[:])
            sd = stpool.tile([P, 1], f32, tag="sd", name="sd")
            nc.scalar.activation(sd[:], mv[:, 1:2], AF.Sqrt, bias=epsc[:], scale=1.0)
            rstd = stpool.tile([P, 1], f32, tag="rstd", name="rstd")
            nc.vector.reciprocal(rstd[:], sd[:])
            if not want_nmr:
                return rstd, None
            nmr = stpool.tile([P, 1], f32, tag="nmr", name="nmr")
            nc.vector.scalar_tensor_tensor(
                nmr[:], mv[:, 0:1], -1.0, rstd[:], op0=ALU.mult, op1=ALU.mult)
            return rstd, nmr

        def layer_norm(src_ap, dst_ap):
            rstd, nmr = ln_stats(src_ap, True)
            nc.scalar.activation(dst_ap, src_ap, AF.Identity,
                                 bias=nmr[:], scale=rstd[:])

        def transpose_v(half):
            for m in range(4 * half, 4 * half + 4):
                for d in range(ND):
                    tps = psA.tile([P, P], f32r, tag="psA", name="tvp")
                    nc.tensor.transpose(tps[:], v_sb[m][:, ts(d, P)], identr[:])
                    nc.vector.tensor_copy(vT_sb[d][:, ts(m, P)], tps[:])  # CAST to bf16

        def emit_xproj(j, half):
            # xT[j][e][:, half] = relu(Dx_j^T @ vT[:, half]); evict alternates
            # ACT/DVE so neither engine paces the 16-matmul burst
            hs = slice(half * HF, (half + 1) * HF)
            for e in range(NDh):
                xps = psA.tile([P, HF], f32, tag="psA", name="xps")
                for k in range(ND):
                    nc.tensor.matmul(xps[:], dx_sb[2 * j + k][:, ts(e, P)],
                                     vT_sb[k][:, hs],
                                     start=(k == 0), stop=(k == ND - 1))
                nc.scalar.activation(xT[j][e][:, hs], xps[:], AF.Relu)

        def emit_rope(j, half):
            # xr = x*cos + rotate_half(x)*sin on [128,512] bf16 slices
            # (all-bf16 SBUF operands -> DVE 2x packed mode)
            hs = slice(half * HF, (half + 1) * HF)
            for m in range(4):
                lo, hi = xT[j][m], xT[j][m + 4]
                xrl, xrh = xr[j][m], xr[j][m + 4]
                cm, sm = cos_sb[m], sin_sb[m]
                t1 = rtpool.tile([P, HF], bf16, tag="rt", name="rt1")
                nc.vector.tensor_mul(t1[:], hi[:, hs], sm[:, hs])
                nc.vector.tensor_mul(xrl[:, hs], lo[:, hs], cm[:, hs])
                nc.vector.tensor_sub(xrl[:, hs], xrl[:, hs], t1[:])
                t2 = rtpool.tile([P, HF], bf16, tag="rt", name="rt2")
                nc.vector.tensor_mul(t2[:], lo[:, hs], sm[:, hs])
                nc.vector.tensor_mul(xrh[:, hs], hi[:, hs], cm[:, hs])
                nc.vector.tensor_add(xrh[:, hs], xrh[:, hs], t2[:])

        def emit_scores_aT(j):
            # S = xr @ xr^T streamed per 128-row tile; aT += v^T @ S with a
            # one-tile lag so the PE never waits on the ACT eviction. S is
            # numerically symmetric so [t,s] tiles serve as [s,t] operands.
            aT_ps = [psB.tile([P, T], f32, tag="psB", name=f"aTps{m}")
                     for m in range(ND)]
            s_tiles = [None] * NT

            def emit_aT(k):
                for m in range(ND):
                    for n in range(2):
                        nc.tensor.matmul(
                            aT_ps[m][:, ts(n, HF)], v_sb[k][:, ts(m, P)],
                            s_tiles[k][:, ts(n, HF)],
                            start=(k == 0), stop=(k == NT - 1))

            for k in range(NT):
                s_sb = spool.tile([P, T], f32r, tag="score", name=f"s{k}")
                for n in range(2):
                    sps = psA.tile([P, HF], f32, tag="psA", name="sps")
                    for kk in range(NDh):
                        nc.tensor.matmul(
                            sps[:], xr[j][kk][:, ts(k, P)],
                            xr[j][kk][:, ts(n, HF)],
                            start=(kk == 0), stop=(kk == NDh - 1))
                    nc.scalar.copy(s_sb[:, ts(n, HF)], sps[:])
                s_tiles[k] = s_sb
                if k > 0:
                    emit_aT(k - 1)
            emit_aT(NT - 1)
            aT = []
            for m in range(ND):
                at = apool.tile([P, T], bf16, tag="aT", name=f"aT{m}")
                aT.append(at)
            # evict in (n, m) order so y's first matmuls (token half 0) start
            # after two of the four half-copies
            for n in range(2):
                for m in range(ND):
                    nc.scalar.copy(aT[m][:, ts(n, HF)], aT_ps[m][:, ts(n, HF)])
            return aT

        def emit_yz_half(j, half, aT, z_ps):
            # yT = relu(Dy^T aT) * xT for one token half; z_ps[:, half]
            # accumulates E_h^T @ yT with a one-tile lag behind the DVE fusion
            hs = slice(half * HF, (half + 1) * HF)
            y_half = [None] * NDh

            def emit_z(kk):
                for m in range(ND):
                    nc.tensor.matmul(
                        z_ps[m][:, hs], eh_sb[8 * j + kk][:, ts(m, P)],
                        y_half[kk][:],
                        start=(kk == 0), stop=(kk == NDh - 1))

            for kk in range(NDh):
                yps = psA.tile([P, HF], f32, tag="psA", name="yps")
                for k in range(ND):
                    nc.tensor.matmul(yps[:], dy_sb[2 * j + k][:, ts(kk, P)],
                                     aT[k][:, hs],
                                     start=(k == 0), stop=(k == ND - 1))
                y_sb = ypool.tile([P, HF], bf16, tag="yT", name=f"y{kk}")
                nc.vector.scalar_tensor_tensor(
                    y_sb[:], yps[:], 0.0, xT[j][kk][:, hs],
                    op0=ALU.max, op1=ALU.mult)
                y_half[kk] = y_sb
                if kk > 0:
                    emit_z(kk - 1)
            emit_z(NDh - 1)

        rg = [[0, 1], [2, 3], [4, 5], [6, 7]]

        # ---- embedding: v0 = LN(onehot @ emb_w) ----
        for m in range(NT):
            eps_t = psA.tile([P, D], f32, tag="psA", name="embps")
            for k in range(ND):
                nc.tensor.matmul(eps_t[:], oh_sb[k][:, ts(m, P)], ew_sb[k][:],
                                 start=(k == 0), stop=(k == ND - 1))
            emb_t = lnpool.tile([P, D], f32, tag="w", name="embt")
            nc.scalar.copy(emb_t[:], eps_t[:])
            layer_norm(emb_t[:], v_sb[m][:])
        transpose_v(0)
        transpose_v(1)
        # layer 0 x-proj + RoPE, head-major so scores h0 unblocks earliest
        emit_xproj(0, 0)
        emit_rope(0, 0)
        emit_xproj(0, 1)
        emit_rope(0, 1)
        emit_xproj(1, 0)
        emit_xproj(1, 1)
        emit_rope(1, 0)
        emit_rope(1, 1)

        for layer in range(L):
            # ---- head 0: scores, aT, y/z; z kept on-chip in bf16 ----
            aT0 = emit_scores_aT(0)
            z0_ps = [psB.tile([P, T], f32, tag="psB", name=f"z0ps{m}")
                     for m in range(ND)]
            emit_yz_half(0, 0, aT0, z0_ps)
            emit_yz_half(0, 1, aT0, z0_ps)
            z0T = [zpool.tile([P, T], bf16, tag=f"z0T{m}", name=f"z0T{m}")
                   for m in range(ND)]
            for m in range(ND):
                nc.scalar.copy(z0T[m][:], z0_ps[m][:])

            # ---- head 1: scores, aT, then y/z per token half; each half's
            # z is fused-added with z0T on eviction and AllReduced over the
            # core pair while later work proceeds ----
            aT1 = emit_scores_aT(1)
            z1_ps = [psB.tile([P, T], f32, tag="psB", name=f"z1ps{m}")
                     for m in range(ND)]
            zin = [None, None]
            zout = [None, None]
            zr = [[None] * ND, [None] * ND]
            for half in range(2):
                hs = slice(half * HF, (half + 1) * HF)
                emit_yz_half(1, half, aT1, z1_ps)
                zin[half] = dpool.tile([P, T], bf16, tag=f"zi{half}",
                                       name=f"zi{half}_{layer}")
                zout[half] = dpool.tile([P, T], bf16, tag=f"zo{half}",
                                        name=f"zo{half}_{layer}")
                for m in range(ND):
                    zq1 = zqpool.tile([P, HF], bf16, tag="zq1",
                                      name=f"zq1_{half}_{m}")
                    nc.vector.scalar_tensor_tensor(
                        zq1[:], z1_ps[m][:, hs], 0.0, z0T[m][:, hs],
                        op0=ALU.add, op1=ALU.add)
                    nc.sync.dma_start(zin[half][:, ts(m, HF)], zq1[:])
                nc.gpsimd.collective_compute(
                    "AllReduce", mybir.AluOpType.add,
                    ins=[zin[half].opt()], outs=[zout[half].opt()],
                    replica_groups=rg)
                for m in range(ND):
                    zrt = zqpool.tile([P, HF], bf16, tag=f"zr{half}{m}",
                                      name=f"zr{half}{m}_{layer}")
                    nc.sync.dma_start(zrt[:], zout[half][:, ts(m, HF)])
                    zr[half][m] = zrt
                if half == 0:
                    keep_warm(20, f"ar{layer}")

            # ---- per token half: v update, then next layer's x-proj/RoPE
            # (or the readout) for that half ----
            for half in range(2):
                for mm in range(4):
                    m = 4 * half + mm
                    q = lnpool.tile([P, D], bf16, tag="zq", name=f"zq{m}")
                    for kd in range(ND):
                        tzp = psA.tile([P, P], bf16, tag="psA", name="tzp")
                        nc.tensor.transpose(
                            tzp[:], zr[half][kd][:, ts(mm, P)], identb[:])
                        nc.vector.tensor_copy(q[:, ts(kd, P)], tzp[:])
                    # v' = LN(v + LN(q)); LN is shift-invariant, so the inner
                    # mean offset can be dropped: w' = q*rstd_q + v differs
                    # from v + LN(q) by a per-token constant only.
                    r_q, _ = ln_stats(q[:], False)
                    wp = lnpool.tile([P, D], f32, tag="w", name=f"w{m}")
                    nc.vector.scalar_tensor_tensor(
                        wp[:], q[:], r_q[:], v_sb[m][:].bitcast(f32),
                        op0=ALU.mult, op1=ALU.add)
                    layer_norm(wp[:], v_sb[m][:])
                    for d in range(ND):
                        tps = psA.tile([P, P], f32r, tag="psA", name="tvp")
                        nc.tensor.transpose(
                            tps[:], v_sb[m][:, ts(d, P)], identr[:])
                        nc.vector.tensor_copy(vT_sb[d][:, ts(m, P)], tps[:])
                    if mm < 3:
                        keep_warm(3, f"ln{layer}_{half}_{mm}")
                if layer == L - 1:
                    for m in range(4 * half, 4 * half + 4):
                        rps = psA.tile([P, V], f32, tag="psA", name="rps")
                        for k in range(ND):
                            nc.tensor.matmul(
                                rps[:], vT_sb[k][:, ts(m, P)], ro_sb[k][:],
                                start=(k == 0), stop=(k == ND - 1))
                        o_sb = lnpool.tile([P, V], f32, tag="o", name=f"o{m}")
                        nc.scalar.copy(o_sb[:], rps[:])
                        nc.sync.dma_start(d_out[ts(m, P), :], o_sb[:])
                else:
                    emit_xproj(0, half)
                    emit_rope(0, half)
                    emit_xproj(1, half)
                    if half == 1:
                        emit_rope(1, 0)
                        emit_rope(1, 1)

    nc.compile()
    return nc


def _get_program():
    if "nc" not in _CACHE:
        _CACHE["nc"] = _build_program()
    return _CACHE["nc"]


def _rope_tables():
    inv = (1.0 / (10000.0 ** (np.arange(0, Dh, 2, dtype=np.float32) / Dh)))
    tt = np.arange(T, dtype=np.float32)
    freqs = np.outer(tt, inv).astype(np.float32)  # [T, Dh/2]
    cosT = np.ascontiguousarray(np.cos(freqs).T, dtype=np.float32)
    sinT = np.ascontiguousarray(np.sin(freqs).T, dtype=np.float32)
    return cosT, sinT


def kernel(**inputs):
    global LAST_RESULT
    from concourse import bass_utils
    import ml_dtypes

    bf = ml_dtypes.bfloat16
    tokens = np.asarray(inputs["tokens"])
    emb_w = np.asarray(inputs["emb_w"], dtype=np.float32).astype(bf)
    E = np.asarray(inputs["E"], dtype=np.float32)
    Dx = np.ascontiguousarray(inputs["Dx"], dtype=np.float32).astype(bf)
    Dy = np.ascontiguousarray(inputs["Dy"], dtype=np.float32).astype(bf)
    readout = np.ascontiguousarray(
        inputs["readout"], dtype=np.float32).astype(bf)

    cosT, sinT = _rope_tables()
    cosT = cosT.astype(bf)
    sinT = sinT.astype(bf)

    in_maps = []
    for c in range(NCORES):
        b, hp = c // 2, c % 2
        oh = np.zeros((V, T), dtype=bf)
        oh[np.asarray(tokens[b], dtype=np.int64), np.arange(T)] = 1.0
        in_maps.append({
            "onehotT": oh,
            "emb_w": emb_w,
            "dx": np.ascontiguousarray(
                Dx[2 * hp:2 * hp + 2].reshape(2 * D, Dh)),
            "dy": np.ascontiguousarray(
                Dy[2 * hp:2 * hp + 2].reshape(2 * D, Dh)),
            "eh": np.ascontiguousarray(
                E[2 * hp * Dh:(2 * hp + 2) * Dh]).astype(bf),
            "cosT": cosT,
            "sinT": sinT,
            "readout": readout,
        })

    nc = _get_program()
    res = bass_utils.run_bass_kernel_spmd(
        nc, in_maps, core_ids=list(range(NCORES)),
        trace=bool(int(os.environ.get("KERNEL_TRACE", "0"))))
    LAST_RESULT = res
    out = np.stack([res.results[2 * b]["out"] for b in range(B)], axis=0)
    return out


# revision 26
# speedup vs baseline: 1.2251x; 1.1391x over previous
# Trainium2 Bass kernel for nn_BDH_66056597013022 (dense_transformer).
#
# Model (per reference):
#   v = LN(emb_w[tokens])                                  [B,T,D]
#   6x: x  = relu(v @ Dx_h)            per head            [B,H,T,Dh]
#       xr = RoPE(x)
#       S  = xr @ xr^T                 (no softmax)        [B,H,T,T]
#       a  = S @ v                                         [B,H,T,D]
#       y  = relu(a @ Dy_h) * x                            [B,H,T,Dh]
#       v  = LN(v + LN(concat_h(y) @ E))
#   out = v @ readout                                      [B,T,V]
#
# Shapes: B=4 T=1024 H=4 N=4096 D=256 L=6 V=256, Dh=N/H=1024.
#
# Sharding (8 cores): core c -> batch b=c//2, head-pair hp=c%2 (heads 2hp,2hp+1).
# Cross-core coupling is only the head-sum of z = y @ E: a 2-rank AllReduce
# per layer between cores {2b,2b+1}, split into two token-half AllReduces so
# the second half's collective overlaps the first half's v-update and the
# next layer's x-projection.
#
# dtypes: v/vT, S, aT, Dx, Dy, readout are float32r (TF32-like, 1 cyc/row);
# xT/xr (RoPE path), y, z/AllReduce path, E, cos/sin, embedding are bf16.
# bf16 gives the DVE 2x packed mode for the RoPE tensor_tensor chain (the
# scores input only; the residual/v chain stays f32r for accuracy) and
# halves the collective payload. PSUM accumulation is fp32 everywhere.
#
# Per-layer pipeline (emission = intended engine order):
#   scores h0 + aT h0 (PE)      [RoPE h1 of this layer ran during prev tail]
#   y/z h0 (PE; STT relu*x on DVE), z0T kept in SBUF
#   scores h1 + aT h1, y/z h1 in token halves; each half: fused add(z0T)+
#     evict -> DMA -> AllReduce (pair) -> DMA back
#   per half: transpose z back to [T,D], LN chain, v update, transpose_v,
#     then NEXT layer's x-proj for that token half + RoPE (so the second
#     half's AllReduce hides behind the first half's x-proj/RoPE).

import os
import numpy as np

B, T, H, N, D, L, V = 4, 1024, 4, 4096, 256, 6, 256
Dh = N // H
EPS = 1e-5
NCORES = 8
P = 128
NT = T // P    # 8 token tiles
ND = D // P    # 2 model-dim tiles
NDh = Dh // P  # 8 head-dim tiles
HF = T // 2    # 512 token half

_CACHE = {}
LAST_RESULT = None


def _build_program():
    from contextlib import ExitStack

    import concourse.bass as bass
    import concourse.bacc as bacc
    import concourse.tile as tile
    import concourse.mybir as mybir
    from concourse.masks import make_identity

    f32 = mybir.dt.float32
    f32r = mybir.dt.float32r
    bf16 = mybir.dt.bfloat16
    AF = mybir.ActivationFunctionType
    ALU = mybir.AluOpType
    ts = bass.ts

    nc = bacc.Bacc("TRN2", target_bir_lowering=False, debug=False,
                   enable_asserts=False, num_devices=NCORES)

    d_oh = nc.dram_tensor("onehotT", [V, T], bf16, kind="ExternalInput").ap()
    d_ew = nc.dram_tensor("emb_w", [V, D], bf16, kind="ExternalInput").ap()
    d_dx = nc.dram_tensor("dx", [2 * D, Dh], bf16, kind="ExternalInput").ap()
    d_dy = nc.dram_tensor("dy", [2 * D, Dh], bf16, kind="ExternalInput").ap()
    d_eh = nc.dram_tensor("eh", [2 * Dh, D], bf16, kind="ExternalInput").ap()
    d_cos = nc.dram_tensor("cosT", [Dh // 2, T], bf16, kind="ExternalInput").ap()
    d_sin = nc.dram_tensor("sinT", [Dh // 2, T], bf16, kind="ExternalInput").ap()
    d_ro = nc.dram_tensor("readout", [D, V], bf16, kind="ExternalInput").ap()
    d_out = nc.dram_tensor("out", [T, V], f32, kind="ExternalOutput").ap()

    with tile.TileContext(nc) as tc, ExitStack() as ctx:
        wpool = ctx.enter_context(tc.tile_pool(name="weights", bufs=1))
        vpool = ctx.enter_context(tc.tile_pool(name="vpool", bufs=1))
        xpool = ctx.enter_context(tc.tile_pool(name="xpool", bufs=1))
        xrpool = ctx.enter_context(tc.tile_pool(name="xrpool", bufs=1))
        spool = ctx.enter_context(tc.tile_pool(name="spool", bufs=4))
        apool = ctx.enter_context(tc.tile_pool(name="apool", bufs=2))
        ypool = ctx.enter_context(tc.tile_pool(name="ypool", bufs=6))
        zpool = ctx.enter_context(tc.tile_pool(name="zpool", bufs=1))
        zqpool = ctx.enter_context(tc.tile_pool(name="zqpool", bufs=4))
        lnpool = ctx.enter_context(tc.tile_pool(name="lnpool", bufs=4))
        stpool = ctx.enter_context(tc.tile_pool(name="stpool", bufs=6))
        rtpool = ctx.enter_context(tc.tile_pool(name="rtpool", bufs=4))
        psA = ctx.enter_context(tc.tile_pool(name="psA", bufs=4, space="PSUM"))
        psB = ctx.enter_context(tc.tile_pool(name="psB", bufs=2, space="PSUM"))
        dpool = ctx.enter_context(tc.tile_pool(name="drampool", bufs=2, space="DRAM"))

        # ---- persistent weights ----
        def load_bf(dram_ap, n_tiles, width, tag):
            tiles = []
            for i in range(n_tiles):
                t = wpool.tile([P, width], bf16, tag=f"{tag}{i}", name=f"{tag}{i}")
                nc.sync.dma_start(t[:], dram_ap[ts(i, P), :])
                tiles.append(t)
            return tiles

        # embedding inputs first so its matmuls start while the big weight
        # DMAs are still in flight
        oh_sb = []
        for k in range(ND):
            t = spool.tile([P, T], bf16, tag="score", name=f"oh{k}")
            nc.sync.dma_start(t[:], d_oh[ts(k, P), :])
            oh_sb.append(t)
        ew_sb = load_bf(d_ew, ND, D, "ew")
        dx_sb = load_bf(d_dx, 4, Dh, "dx")
        cos_sb = load_bf(d_cos, 4, T, "cos")
        sin_sb = load_bf(d_sin, 4, T, "sin")
        dy_sb = load_bf(d_dy, 4, Dh, "dy")
        eh_sb = load_bf(d_eh, 16, D, "eh")
        ro_sb = load_bf(d_ro, ND, V, "ro")

        identf = wpool.tile([P, P], f32, tag="identf", name="identf")
        make_identity(nc, identf)
        identr = wpool.tile([P, P], f32r, tag="identr", name="identr")
        nc.scalar.copy(identr[:], identf[:])
        identb = wpool.tile([P, P], bf16, tag="identb", name="identb")
        nc.scalar.copy(identb[:], identf[:])
        epsc = wpool.tile([P, 1], f32, tag="epsc", name="epsc")
        nc.gpsimd.memset(epsc[:], EPS)
        warmsink = wpool.tile([P, 1], f32, tag="warmsink", name="warmsink")

        def keep_warm(n_mms, label):
            # dependency-free matmuls to hold the PE clock at 2.4 GHz across
            # a known stall (the AllReduce tail / LN chains leave PE idle
            # long enough for the HAM to re-throttle to 1.2 GHz otherwise)
            wps = psA.tile([P, 512], f32, tag="psA", name=f"warm_{label}")
            for i in range(n_mms):
                nc.tensor.matmul(wps[:], dx_sb[0][:, 0:P], dx_sb[1][:, 0:512],
                                 start=(i == 0), stop=(i == n_mms - 1))
            nc.scalar.copy(warmsink[:], wps[:, 0:1])

        # ---- persistent activations ----
        v_sb = [vpool.tile([P, D], f32r, tag=f"v{m}", name=f"v{m}")
                for m in range(NT)]
        vT_sb = [vpool.tile([P, T], bf16, tag=f"vT{k}", name=f"vT{k}")
                 for k in range(ND)]
        # xT/xr: 2 heads x 8 Dh-tiles, persistent slots reused per layer
        xT = [[xpool.tile([P, T], bf16, tag=f"xT{j}_{e}", name=f"xT{j}_{e}")
               for e in range(NDh)] for j in range(2)]
        xr = [[xrpool.tile([P, T], bf16, tag=f"xr{j}_{e}", name=f"xr{j}_{e}")
               for e in range(NDh)] for j in range(2)]

        def ln_stats(src_ap, want_nmr):
            # rstd (and optionally -mean*rstd) of src over the free dim
            st6 = stpool.tile([P, 6], f32, tag="st6", name="st6")
            nc.vector.bn_stats(st6[:], src_ap)
            mv = stpool.tile([P, 2], f32, tag="mv", name="mv")
            nc.vector.bn_aggr(mv[:], st6[:])
            sd = stpool.tile([P, 1], f32, tag="sd", name="sd")
            nc.scalar.activation(sd[:], mv[:, 1:2], AF.Sqrt, bias=epsc[:], scale=1.0)
            rstd = stpool.tile([P, 1], f32, tag="rstd", name="rstd")
            nc.vector.reciprocal(rstd[:], sd[:])
            if not want_nmr:
                return rstd, None
            nmr = stpool.tile([P, 1], f32, tag="nmr", name="nmr")
            nc.vector.scalar_tensor_tensor(
                nmr[:], mv[:, 0:1], -1.0, rstd[:], op0=ALU.mult, op1=ALU.mult)
            return rstd, nmr

        def layer_norm(src_ap, dst_ap):
            rstd, nmr = ln_stats(src_ap, True)
            nc.scalar.activation(dst_ap, src_ap, AF.Identity,
                                 bias=nmr[:], scale=rstd[:])

        def transpose_v(half):
            for m in range(4 * half, 4 * half + 4):
                for d in range(ND):
                    tps = psA.tile([P, P], f32r, tag="psA", name="tvp")
                    nc.tensor.transpose(tps[:], v_sb[m][:, ts(d, P)], identr[:])
                    nc.vector.tensor_copy(vT_sb[d][:, ts(m, P)], tps[:])  # CAST to bf16

        def emit_xproj(j, half):
            # xT[j][e][:, half] = relu(Dx_j^T @ vT[:, half]); evict alternates
            # ACT/DVE so neither engine paces the 16-matmul burst
            hs = slice(half * HF, (half + 1) * HF)
            for e in range(NDh):
                xps = psA.tile([P, HF], f32, tag="psA", name="xps")
                for k in range(ND):
                    nc.tensor.matmul(xps[:], dx_sb[2 * j + k][:, ts(e, P)],
                                     vT_sb[k][:, hs],
                                     start=(k == 0), stop=(k == ND - 1))
                nc.scalar.activation(xT[j][e][:, hs], xps[:], AF.Relu)

        def emit_rope(j, half):
            # xr = x*cos + rotate_half(x)*sin on [128,512] bf16 slices
            # (all-bf16 SBUF operands -> DVE 2x packed mode)
            hs = slice(half * HF, (half + 1) * HF)
            for m in range(4):
                lo, hi = xT[j][m], xT[j][m + 4]
                xrl, xrh = xr[j][m], xr[j][m + 4]
                cm, sm = cos_sb[m], sin_sb[m]
                t1 = rtpool.tile([P, HF], bf16, tag="rt", name="rt1")
                nc.vector.tensor_mul(t1[:], hi[:, hs], sm[:, hs])
                nc.vector.tensor_mul(xrl[:, hs], lo[:, hs], cm[:, hs])
                nc.vector.tensor_sub(xrl[:, hs], xrl[:, hs], t1[:])
                t2 = rtpool.tile([P, HF], bf16, tag="rt", name="rt2")
                nc.vector.tensor_mul(t2[:], lo[:, hs], sm[:, hs])
                nc.vector.tensor_mul(xrh[:, hs], hi[:, hs], cm[:, hs])
                nc.vector.tensor_add(xrh[:, hs], xrh[:, hs], t2[:])

        def emit_scores_aT(j):
            # S = xr @ xr^T streamed per 128-row tile; aT += v^T @ S with a
            # one-tile lag so the PE never waits on the ACT eviction. S is
            # numerically symmetric so [t,s] tiles serve as [s,t] operands.
            aT_ps = [psB.tile([P, T], f32, tag="psB", name=f"aTps{m}")
                     for m in range(ND)]
            s_tiles = [None] * NT

            def emit_aT(k):
                for m in range(ND):
                    for n in range(2):
                        nc.tensor.matmul(
                            aT_ps[m][:, ts(n, HF)], v_sb[k][:, ts(m, P)],
                            s_tiles[k][:, ts(n, HF)],
                            start=(k == 0), stop=(k == NT - 1))

            for k in range(NT):
                s_sb = spool.tile([P, T], f32r, tag="score", name=f"s{k}")
                for n in range(2):
                    sps = psA.tile([P, HF], f32, tag="psA", name="sps")
                    for kk in range(NDh):
                        nc.tensor.matmul(
                            sps[:], xr[j][kk][:, ts(k, P)],
                            xr[j][kk][:, ts(n, HF)],
                            start=(kk == 0), stop=(kk == NDh - 1))
                    nc.scalar.copy(s_sb[:, ts(n, HF)], sps[:])
                s_tiles[k] = s_sb
                if k > 0:
                    emit_aT(k - 1)
            emit_aT(NT - 1)
            aT = []
            for m in range(ND):
                at = apool.tile([P, T], bf16, tag="aT", name=f"aT{m}")
                aT.append(at)
            # evict in (n, m) order so y's first matmuls (token half 0) start
            # after two of the four half-copies
            for n in range(2):
                for m in range(ND):
                    nc.scalar.copy(aT[m][:, ts(n, HF)], aT_ps[m][:, ts(n, HF)])
            return aT

        def emit_yz_half(j, half, aT, z_ps):
            # yT = relu(Dy^T aT) * xT for one token half; z_ps[:, half]
            # accumulates E_h^T @ yT with a one-tile lag behind the DVE fusion
            hs = slice(half * HF, (half + 1) * HF)
            y_half = [None] * NDh

            def emit_z(kk):
                for m in range(ND):
                    nc.tensor.matmul(
                        z_ps[m][:, hs], eh_sb[8 * j + kk][:, ts(m, P)],
                        y_half[kk][:],
                        start=(kk == 0), stop=(kk == NDh - 1))

            for kk in range(NDh):
                yps = psA.tile([P, HF], f32, tag="psA", name="yps")
                for k in range(ND):
                    nc.tensor.matmul(yps[:], dy_sb[2 * j + k][:, ts(kk, P)],
                                     aT[k][:, hs],
                                     start=(k == 0), stop=(k == ND - 1))
                y_sb = ypool.tile([P, HF], bf16, tag="yT", name=f"y{kk}")
                nc.vector.scalar_tensor_tensor(
                    y_sb[:], yps[:], 0.0, xT[j][kk][:, hs],
                    op0=ALU.max, op1=ALU.mult)
                y_half[kk] = y_sb
                if kk > 0:
                    emit_z(kk - 1)
            emit_z(NDh - 1)

        rg = [[0, 1], [2, 3], [4, 5], [6, 7]]

        # ---- embedding: v0 = LN(onehot @ emb_w) ----
        for m in range(NT):
            eps_t = psA.tile([P, D], f32, tag="psA", name="embps")
            for k in range(ND):
                nc.tensor.matmul(eps_t[:], oh_sb[k][:, ts(m, P)], ew_sb[k][:],
                                 start=(k == 0), stop=(k == ND - 1))
            emb_t = lnpool.tile([P, D], f32, tag="w", name="embt")
            nc.scalar.copy(emb_t[:], eps_t[:])
            layer_norm(emb_t[:], v_sb[m][:])
        transpose_v(0)
        transpose_v(1)
        # layer 0 x-proj + RoPE, head-major so scores h0 unblocks earliest
        emit_xproj(0, 0)
        emit_rope(0, 0)
        emit_xproj(0, 1)
        emit_rope(0, 1)
        emit_xproj(1, 0)
        emit_xproj(1, 1)
        emit_rope(1, 0)
        emit_rope(1, 1)

        for layer in range(L):
            # ---- head 0: scores, aT, y/z; z kept on-chip in bf16 ----
            aT0 = emit_scores_aT(0)
            z0_ps = [psB.tile([P, T], f32, tag="psB", name=f"z0ps{m}")
                     for m in range(ND)]
            emit_yz_half(0, 0, aT0, z0_ps)
            emit_yz_half(0, 1, aT0, z0_ps)
            z0T = [zpool.tile([P, T], bf16, tag=f"z0T{m}", name=f"z0T{m}")
                   for m in range(ND)]
            for m in range(ND):
                nc.scalar.copy(z0T[m][:], z0_ps[m][:])

            # ---- head 1: scores, aT, then y/z per token half; each half's
            # z is fused-added with z0T on eviction and AllReduced over the
            # core pair while later work proceeds ----
            aT1 = emit_scores_aT(1)
            z1_ps = [psB.tile([P, T], f32, tag="psB", name=f"z1ps{m}")
                     for m in range(ND)]
            zin = [None, None]
            zout = [None, None]
            zr = [[None] * ND, [None] * ND]
            for half in range(2):
                hs = slice(half * HF, (half + 1) * HF)
                emit_yz_half(1, half, aT1, z1_ps)
                zin[half] = dpool.tile([P, T], bf16, tag=f"zi{half}",
                                       name=f"zi{half}_{layer}")
                zout[half] = dpool.tile([P, T], bf16, tag=f"zo{half}",
                                        name=f"zo{half}_{layer}")
                for m in range(ND):
                    zq1 = zqpool.tile([P, HF], bf16, tag="zq1",
                                      name=f"zq1_{half}_{m}")
                    nc.vector.scalar_tensor_tensor(
                        zq1[:], z1_ps[m][:, hs], 0.0, z0T[m][:, hs],
                        op0=ALU.add, op1=ALU.add)
                    nc.sync.dma_start(zin[half][:, ts(m, HF)], zq1[:])
                nc.gpsimd.collective_compute(
                    "AllReduce", mybir.AluOpType.add,
                    ins=[zin[half].opt()], outs=[zout[half].opt()],
                    replica_groups=rg)
                for m in range(ND):
                    zrt = zqpool.tile([P, HF], bf16, tag=f"zr{half}{m}",
                                      name=f"zr{half}{m}_{layer}")
                    nc.sync.dma_start(zrt[:], zout[half][:, ts(m, HF)])
                    zr[half][m] = zrt
                if half == 0:
                    keep_warm(20, f"ar{layer}")

            # ---- per token half: v update, then next layer's x-proj/RoPE
            # (or the readout) for that half ----
            for half in range(2):
                for mm in range(4):
                    m = 4 * half + mm
                    q = lnpool.tile([P, D], bf16, tag="zq", name=f"zq{m}")
                    for kd in range(ND):
                        tzp = psA.tile([P, P], bf16, tag="psA", name="tzp")
                        nc.tensor.transpose(
                            tzp[:], zr[half][kd][:, ts(mm, P)], identb[:])
                        nc.vector.tensor_copy(q[:, ts(kd, P)], tzp[:])
                    # v' = LN(v + LN(q)); LN is shift-invariant, so the inner
                    # mean offset can be dropped: w' = q*rstd_q + v differs
                    # from v + LN(q) by a per-token constant only.
                    r_q, _ = ln_stats(q[:], False)
                    wp = lnpool.tile([P, D], f32, tag="w", name=f"w{m}")
                    nc.vector.scalar_tensor_tensor(
                        wp[:], q[:], r_q[:], v_sb[m][:].bitcast(f32),
                        op0=ALU.mult, op1=ALU.add)
                    layer_norm(wp[:], v_sb[m][:])
                transpose_v(half)
                if layer == L - 1:
                    for m in range(4 * half, 4 * half + 4):
                        rps = psA.tile([P, V], f32, tag="psA", name="rps")
                        for k in range(ND):
                            nc.tensor.matmul(
                                rps[:], vT_sb[k][:, ts(m, P)], ro_sb[k][:],
                                start=(k == 0), stop=(k == ND - 1))
                        o_sb = lnpool.tile([P, V], f32, tag="o", name=f"o{m}")
                        nc.scalar.copy(o_sb[:], rps[:])
                        nc.sync.dma_start(d_out[ts(m, P), :], o_sb[:])
                else:
                    emit_xproj(0, half)
                    emit_rope(0, half)
                    emit_xproj(1, half)
                    if half == 1:
                        emit_rope(1, 0)
                        emit_rope(1, 1)

    nc.compile()
    return nc


def _get_program():
    if "nc" not in _CACHE:
        _CACHE["nc"] = _build_program()
    return _CACHE["nc"]


def _rope_tables():
    inv = (1.0 / (10000.0 ** (np.arange(0, Dh, 2, dtype=np.float32) / Dh)))
    tt = np.arange(T, dtype=np.float32)
    freqs = np.outer(tt, inv).astype(np.float32)  # [T, Dh/2]
    cosT = np.ascontiguousarray(np.cos(freqs).T, dtype=np.float32)
    sinT = np.ascontiguousarray(np.sin(freqs).T, dtype=np.float32)
    return cosT, sinT


def kernel(**inputs):
    global LAST_RESULT
    from concourse import bass_utils
    import ml_dtypes

    bf = ml_dtypes.bfloat16
    tokens = np.asarray(inputs["tokens"])
    emb_w = np.asarray(inputs["emb_w"], dtype=np.float32).astype(bf)
    E = np.asarray(inputs["E"], dtype=np.float32)
    Dx = np.ascontiguousarray(inputs["Dx"], dtype=np.float32).astype(bf)
    Dy = np.ascontiguousarray(inputs["Dy"], dtype=np.float32).astype(bf)
    readout = np.ascontiguousarray(
        inputs["readout"], dtype=np.float32).astype(bf)

    cosT, sinT = _rope_tables()
    cosT = cosT.astype(bf)
    sinT = sinT.astype(bf)

    in_maps = []
    for c in range(NCORES):
        b, hp = c // 2, c % 2
        oh = np.zeros((V, T), dtype=bf)
        oh[np.asarray(tokens[b], dtype=np.int64), np.arange(T)] = 1.0
        in_maps.append({
            "onehotT": oh,
            "emb_w": emb_w,
            "dx": np.ascontiguousarray(
                Dx[2 * hp:2 * hp + 2].reshape(2 * D, Dh)),
            "dy": np.ascontiguousarray(
                Dy[2 * hp:2 * hp + 2].reshape(2 * D, Dh)),
            "eh": np.ascontiguousarray(
                E[2 * hp * Dh:(2 * hp + 2) * Dh]).astype(bf),
            "cosT": cosT,
            "sinT": sinT,
            "readout": readout,
        })

    nc = _get_program()
    res = bass_utils.run_bass_kernel_spmd(
        nc, in_maps, core_ids=list(range(NCORES)),
        trace=bool(int(os.environ.get("KERNEL_TRACE", "0"))))
    LAST_RESULT = res
    out = np.stack([res.results[2 * b]["out"] for b in range(B)], axis=0)
    return out
